# revision 1
# baseline (speedup 1.0000x reference)
"""Trainium2 Bass kernel for batched multi-head self-attention.

Reference computation (per batch element b):
    qkv = x @ w_qkv.T                  # [N, 3C]
    q, k, v = split/reshape to heads   # H=16 heads, d=64
    attn = softmax(q @ k.T / sqrt(d))
    out = (attn @ v) reshaped back     # [N, C]
    y = out @ w_proj.T + b_proj

Sharding: pure data-parallel over batch B=8 across the 8 NeuronCores
(one batch element per core, weights replicated, no collectives).

On-device layout (everything transposed so matmuls contract over the
partition axis with no on-device transposes):
  - xT      [C, N]   (host pre-transposed, bf16)
  - wqkvT   [C, 3C]  (host pre-transposed, bf16)
  - wprojT  [C, C]   (host pre-transposed, bf16)
  - scores computed as S^T tiles [m, n]; softmax row-sums obtained by
    appending a ones-column to V in the attn@V matmul (PE computes the
    sums for free); normalization applied at PSUM evacuation.

Performance structure (TimelineSim: 229.2us, PE 96.5% busy; the
all-matmul floor is 218.4us = 1024 MMs x 213ns):
  - bf16 matmuls everywhere (fp32 is 4x slower on the PE); fp32 PSUM
    accumulation and fp32 softmax scores keep rel err ~6e-3.
  - QK^T head pairs are row-packed via tile_position (K=64 each, rows
    0-63 / 64-127) -- concurrent on silicon.
  - Emission interleaves attention (ACT-heavy) with the q/k projection
    (PE-heavy) per head-pair so the scheduler fills softmax-bound PE
    bubbles with projection matmuls; output projection is emitted
    n2-outer to fill the last pair's tail.
  - PSUM budget (8 banks): acc 3 + st 3 + av 2; attention loops
    n2-outer so only one n2's AV accumulators are live. Phases whose
    natural tags are idle borrow the other tags' slots (v-projection
    rotates across all three; the final projection chains spread over
    all 8 slots so their preludes run before the last pair lands).
  - Dummy warm-up matmuls during the initial DMA wait complete the
    PE p-state/HAM ramp before real work arrives.
  - x^T and the v-columns of w_qkv are fused host-side into one "xw"
    tensor: one DMA per contraction tile (DMA-start overhead is a
    fixed cost per descriptor).
  - Output stored bf16 (halves store transfers incl. the tail-critical
    one); host converts back to f32. Adds ~0.2% RMS quantization --
    total rel err 6.0e-3 vs the 2e-2 gate.
"""

import os
import sys

for _p in ("/opt/trn_rl_repo", "/root/.axon_site/_ro/trn_rl_repo"):
    if os.path.isdir(_p) and _p not in sys.path:
        sys.path.insert(0, _p)
        break

import numpy as np
import ml_dtypes

import concourse.bass as bass
import concourse.bacc as bacc
import concourse.tile as tile
import concourse.mybir as mybir
from concourse import bass_utils

BF16 = mybir.dt.bfloat16
F32 = mybir.dt.float32
AF = mybir.ActivationFunctionType

B, N, C, H = 8, 1024, 1024, 16
D = C // H            # 64 head dim
P = 128               # partitions
CT = C // P           # 8 contraction tiles
NT2 = N // 512        # 2 n-tiles of 512
MT = N // P           # 8 m-tiles of 128
PAIRS = H // 2        # 8 head pairs
SCALE = float(D) ** -0.5
N_CORES = 8

_cache = {}


def _build():
    nc = bacc.Bacc("TRN2", target_bir_lowering=False, debug=False,
                   enable_asserts=False, num_devices=N_CORES)

    xw_d = nc.dram_tensor("xw", [C, 2 * N], BF16, kind="ExternalInput")
    wqkvT_d = nc.dram_tensor("wqkvT", [C, 3 * C], BF16, kind="ExternalInput")
    wprojT_d = nc.dram_tensor("wprojT", [C, C], BF16, kind="ExternalInput")
    bias_d = nc.dram_tensor("bias", [P, CT], F32, kind="ExternalInput")
    outT_d = nc.dram_tensor("outT", [C, N], BF16, kind="ExternalOutput")

    with tile.TileContext(nc) as tc:
        with (
            tc.tile_pool(name="res", bufs=1) as rp,
            tc.tile_pool(name="work", bufs=2) as wp,
            tc.tile_pool(name="ps", bufs=1, space="PSUM") as pp,
        ):
            # ---------------- PE warm-up ----------------
            # The PE sits idle ~3.5us waiting for the first input DMAs, and
            # the p-state/HAM ramp then penalizes the first ~3us of real
            # matmuls. Run dummy matmuls on memset data during the DMA wait
            # so the ramp completes before real work arrives.
            warm_a = wp.tile([P, 512], BF16, name="warm_a", tag="warm_a",
                             bufs=1)
            nc.gpsimd.memset(warm_a[:], 0.25)
            warm_ps = pp.tile([P, 512], F32, name="warm_ps", tag="acc",
                              bufs=3)
            for _ in range(6):
                nc.tensor.matmul(warm_ps[:], warm_a[:, 0:P], warm_a[:],
                                 start=True, stop=True)

            # ---------------- resident inputs ----------------
            # v-part inputs first so the first matmuls can start ASAP.
            # x and the v-columns of w_qkv are fused host-side into one
            # tensor: one DMA per c-tile (the HWDGE queue stage costs a
            # fixed ~625ns per DMA, so fewer DMAs = faster start)
            xT = []
            wqv = []
            for i in range(CT):
                t = rp.tile([P, 2 * N], BF16, name=f"xw{i}", tag=f"xw{i}")
                nc.sync.dma_start(t[:], xw_d.ap()[i * P:(i + 1) * P, :])
                xT.append(t[:, 0:N])
                wqv.append(t[:, N:2 * N])
            wqk = []
            for i in range(CT):
                t = rp.tile([P, 2 * C], BF16, name=f"wqk{i}", tag=f"wqk{i}")
                nc.sync.dma_start(t[:],
                                  wqkvT_d.ap()[i * P:(i + 1) * P, 0:2 * C])
                wqk.append(t)
            wpj = []
            for i in range(CT):
                t = rp.tile([P, C], BF16, name=f"wpj{i}", tag=f"wpj{i}")
                nc.sync.dma_start(t[:], wprojT_d.ap()[i * P:(i + 1) * P, :])
                wpj.append(t)
            bias_t = rp.tile([P, CT], F32, name="bias_t", tag="bias")
            nc.sync.dma_start(bias_t[:], bias_d.ap())

            # ---------------- result tiles ----------------
            qT = [rp.tile([P, N], BF16, name=f"qT{i}", tag=f"qT{i}")
                  for i in range(PAIRS)]
            kT = [rp.tile([P, N], BF16, name=f"kT{i}", tag=f"kT{i}")
                  for i in range(PAIRS)]
            vt = [[rp.tile([P, 8, D + 1], BF16, name=f"v{m}_{j}",
                           tag=f"v{m}_{j}") for j in range(2)]
                  for m in range(MT)]
            ao = [rp.tile([P, N], BF16, name=f"ao{i}", tag=f"ao{i}")
                  for i in range(PAIRS)]

            for m in range(MT):
                for j in range(2):
                    nc.vector.memset(vt[m][j][:, :, D:D + 1], 1.0)

            # ---------------- phase A(v): v projection ----------------
            # Alternate psum tags: the attention-phase "av" slots are idle
            # here, so borrow them for 4-deep accumulator pipelining.
            for m in range(MT):
                for j in range(2):
                    vtag, vbufs = (("acc", 3), ("av", 2),
                                   ("st", 3))[(2 * m + j) % 3]
                    ps = pp.tile([P, 512], F32, name=f"accv{m}_{j}", tag=vtag,
                                 bufs=vbufs)
                    for c in range(CT):
                        nc.tensor.matmul(
                            ps[:],
                            xT[c][:, m * P:(m + 1) * P],
                            wqv[c][:, j * 512:(j + 1) * 512],
                            start=(c == 0), stop=(c == CT - 1),
                        )
                    nc.vector.tensor_copy(
                        vt[m][j][:, :, 0:D],
                        ps[:].rearrange("p (h d) -> p h d", d=D),
                    )

            # ------- interleaved: q/k projection + attention per pair -------
            for pr in range(PAIRS):
                for which, dst in ((0, qT[pr]), (1, kT[pr])):
                    o0 = which * C + pr * P
                    for n2 in range(NT2):
                        nsl = slice(n2 * 512, (n2 + 1) * 512)
                        ps = pp.tile([P, 512], F32,
                                     name=f"accqk{pr}_{which}_{n2}",
                                     tag="acc", bufs=3)
                        for c in range(CT):
                            nc.tensor.matmul(
                                ps[:],
                                wqk[c][:, o0:o0 + P],
                                xT[c][:, nsl],
                                start=(c == 0), stop=(c == CT - 1),
                            )
                        nc.vector.tensor_copy(dst[:, nsl], ps[:])

                # attention for this pair (n2-outer so only one n2's AV
                # accumulators are live; PSUM budget: acc 3 + st 3 + av 2 = 8)
                for n2 in range(NT2):
                    nsl = slice(n2 * 512, (n2 + 1) * 512)
                    av = [pp.tile([D + 1, 512], F32, name=f"av{pr}_{n2}_{h}",
                                  tag="av", bufs=2) for h in range(2)]
                    for m in range(MT):
                        msl = slice(m * P, (m + 1) * P)
                        st = [pp.tile([P, 512], F32,
                                      name=f"st{pr}_{m}_{n2}_{h}", tag="st",
                                      bufs=3) for h in range(2)]
                        for h in range(2):
                            psl = slice(h * 64, (h + 1) * 64)
                            nc.tensor.matmul(
                                st[h][:],
                                kT[pr][psl, msl],
                                qT[pr][psl, nsl],
                                start=True, stop=True,
                                tile_position=(h * 64, 0),
                            )
                        for h in range(2):
                            pt = wp.tile([P, 512], BF16,
                                         name=f"pt{pr}_{m}_{n2}_{h}",
                                         tag="pt", bufs=6)
                            nc.scalar.activation(pt[:], st[h][:], AF.Exp,
                                                 scale=SCALE)
                            head = 2 * pr + h
                            vtile = vt[m][head // 8]
                            nc.tensor.matmul(
                                av[h][:],
                                vtile[:, head % 8, :],
                                pt[:],
                                start=(m == 0), stop=(m == MT - 1),
                            )
                    # normalize + evacuate this n2 slice. Copy PSUM out
                    # first so the av bank frees fast; normalize from SBUF.
                    # For the very last slice the bank release doesn't
                    # matter; read PSUM directly to shorten the tail chain.
                    last_slice = (pr == PAIRS - 1 and n2 == NT2 - 1)
                    for h in range(2):
                        if last_slice:
                            araw = av[h]
                        else:
                            araw = wp.tile([D + 1, 512], F32,
                                           name=f"araw{pr}_{h}_{n2}",
                                           tag="araw", bufs=4)
                            nc.vector.tensor_copy(araw[:], av[h][:])
                        rec = wp.tile([D + 1, 512], F32,
                                      name=f"rec{pr}_{h}_{n2}", tag="rec",
                                      bufs=4)
                        nc.vector.reciprocal(rec[D:D + 1, :],
                                             araw[D:D + 1, :])
                        rec0 = wp.tile([1, 512], F32,
                                       name=f"rec0_{pr}_{h}_{n2}", tag="rec0",
                                       bufs=4)
                        nc.sync.dma_start(rec0[:], rec[D:D + 1, :])
                        bc = wp.tile([D, 512], F32, name=f"bc{pr}_{h}_{n2}",
                                     tag="bc", bufs=4)
                        nc.gpsimd.partition_broadcast(bc[:], rec0[:])
                        if h == 0:
                            nc.vector.tensor_mul(ao[pr][0:D, nsl],
                                                 araw[0:D, :], bc[:])
                        else:
                            tmp = wp.tile([D, 512], BF16,
                                          name=f"aotmp{pr}_{n2}", tag="aotmp",
                                          bufs=4)
                            nc.vector.tensor_mul(tmp[:], araw[0:D, :],
                                                 bc[:])
                            nc.sync.dma_start(ao[pr][D:P, nsl], tmp[:])

            # ---------------- phase C: output projection ----------------
            # n2-outer: proj over n2=0 becomes ready while the last pair's
            # n2=1 attention still runs, filling the PE tail gap.
            for n2 in range(NT2):
                for ot in range(CT):
                    nsl = slice(n2 * 512, (n2 + 1) * 512)
                    # n2=1 runs at the very end when the attention's st/av
                    # slots are dead: spread the 8 chains across all tags so
                    # every pair-0..6 prelude can run before ao[7] arrives
                    if n2 == 0:
                        ptag, pbufs = "acc", 3
                    else:
                        ptag, pbufs = (("acc", 3), ("st", 3), ("av", 2),
                                       ("acc", 3), ("st", 3), ("av", 2),
                                       ("acc", 3), ("st", 3))[ot]
                    ps = pp.tile([P, 512], F32, name=f"accy{ot}_{n2}",
                                 tag=ptag, bufs=pbufs)
                    for pr in range(PAIRS):
                        nc.tensor.matmul(
                            ps[:],
                            wpj[pr][:, ot * P:(ot + 1) * P],
                            ao[pr][:, nsl],
                            start=(pr == 0), stop=(pr == PAIRS - 1),
                        )
                    yt = wp.tile([P, 512], BF16, name=f"y{ot}_{n2}", tag="y",
                                 bufs=3)
                    nc.vector.tensor_scalar_add(yt[:], ps[:],
                                                bias_t[:, ot:ot + 1])
                    nc.sync.dma_start(outT_d.ap()[ot * P:(ot + 1) * P, nsl],
                                      yt[:])

    nc.compile()
    return nc


def get_nc():
    if "nc" not in _cache:
        _cache["nc"] = _build()
    return _cache["nc"]


def kernel(x, w_qkv, w_proj, b_proj):
    x = np.asarray(x, dtype=np.float32)
    w_qkv = np.asarray(w_qkv, dtype=np.float32)
    w_proj = np.asarray(w_proj, dtype=np.float32)
    b_proj = np.asarray(b_proj, dtype=np.float32)

    bf = ml_dtypes.bfloat16
    wqkvT = np.ascontiguousarray(w_qkv.T).astype(bf)     # [C, 3C]
    wprojT = np.ascontiguousarray(w_proj.T).astype(bf)   # [C, C]
    bias = np.ascontiguousarray(b_proj.reshape(CT, P).T).astype(np.float32)

    in_maps = []
    wqv_host = wqkvT[:, 2 * C:]                          # [C, C] v columns
    for b in range(N_CORES):
        xT = np.ascontiguousarray(x[b].T).astype(bf)     # [C, N]
        xw = np.ascontiguousarray(np.concatenate([xT, wqv_host], axis=1))
        in_maps.append({"xw": xw, "wqkvT": wqkvT, "wprojT": wprojT,
                        "bias": bias})

    nc = get_nc()
    _cache["in_maps"] = in_maps
    res = bass_utils.run_bass_kernel_spmd(nc, in_maps,
                                          core_ids=list(range(N_CORES)))
    out = np.empty((B, N, C), dtype=np.float32)
    for b in range(N_CORES):
        out[b] = res.results[b]["outT"].T.astype(np.float32)
    return out

